# revision 5
# baseline (speedup 1.0000x reference)
"""Multi-head attention (B=4, N=2048, DIM=1024, H=16, DH=64) on 8 trn2 cores.

Sharding: core c handles batch c//2 and head-half c%2 (8 heads).  Each core
computes qkv projection for its heads, attention, and a partial output
projection; the host sums the two partials per batch and adds the bias.
No cross-core collectives needed.

Layout strategy (zero on-device transposes):
  - host supplies x[b] pre-transposed (xT: [DIM, N]) in bf16
  - qT/kT computed as [d, n] ("transposed") via out = W^T @ x^T matmuls
  - S^T tiles [j=128, i=512] from row-packed matmuls (d=64 contraction,
    2 heads concurrently in PE row groups 0-63 / 64-127)
  - exp via ACT (scale folded), PSUM -> SBUF bf16 (P^T tiles)
  - PV: O^T[d, i] += V[j, d]^T-matmul, col-packed pairs (PE col groups)
  - denominators: DVE add-tree over j-tiles + gpsimd partition_all_reduce
  - output projection consumes O^T tiles directly as lhsT
"""

import numpy as np
import ml_dtypes

B, N, DIM = 4, 2048, 1024
HEADS, DH = 16, 64
SCALE = DIM ** (-0.5)
HPC = 8              # heads per core
NPAIR = HPC // 2     # 4 head pairs
CPC = HPC * DH       # 512 channels per core
IB = 512             # i-block (query cols per attention unit)
NIB = N // IB        # 4
NJT = N // 128       # 16 j-tiles
NKT = DIM // 128     # 8 contraction tiles for projections

_cache = {}


def _build():
    import concourse.bacc as bacc
    import concourse.mybir as mybir
    import concourse.tile as tile

    f32 = mybir.dt.float32
    bf16 = mybir.dt.bfloat16

    nc = bacc.Bacc("TRN2", target_bir_lowering=False, debug=False,
                   enable_asserts=False, num_devices=8)

    xT_d = nc.dram_tensor("xT", (DIM, N), bf16, kind="ExternalInput").ap()
    wqkv_d = nc.dram_tensor("wqkv", (DIM, 3 * CPC), bf16, kind="ExternalInput").ap()
    wout_d = nc.dram_tensor("wout", (CPC, DIM), bf16, kind="ExternalInput").ap()
    out_d = nc.dram_tensor("out", (N, DIM), f32, kind="ExternalOutput").ap()

    with tile.TileContext(nc) as tc:
        _body(nc, tc, mybir, xT_d, wqkv_d, wout_d, out_d)

    nc.compile()
    return nc


def _body(nc, tc, mybir, xT_d, wqkv_d, wout_d, out_d):
    import concourse.bass_isa as bass_isa
    from contextlib import ExitStack

    f32 = mybir.dt.float32
    bf16 = mybir.dt.bfloat16
    Exp = mybir.ActivationFunctionType.Exp
    mult = mybir.AluOpType.mult
    add = mybir.AluOpType.add

    ctx = ExitStack()
    with ctx:
        wpool = ctx.enter_context(tc.tile_pool(name="weights", bufs=1))
        qkv_pool = ctx.enter_context(tc.tile_pool(name="qkv", bufs=1))
        ppool = ctx.enter_context(tc.tile_pool(name="ptiles", bufs=2))
        spool = ctx.enter_context(tc.tile_pool(name="small", bufs=3))
        opool = ctx.enter_context(tc.tile_pool(name="oT", bufs=10))
        outp = ctx.enter_context(tc.tile_pool(name="outstage", bufs=3))
        psum2 = ctx.enter_context(tc.tile_pool(name="psum2", bufs=2, space="PSUM"))
        psum = ctx.enter_context(tc.tile_pool(name="psum1", bufs=1, space="PSUM"))
        psum1s = ctx.enter_context(tc.tile_pool(name="psum1s", bufs=1, space="PSUM"))

        # ---- load weights ----
        wqkv_sb = wpool.tile([128, NKT, 3 * CPC], bf16)
        nc.sync.dma_start(wqkv_sb, wqkv_d.rearrange("(ko p) c -> p ko c", p=128))
        wout_sb = wpool.tile([128, NPAIR, DIM], bf16)
        nc.sync.dma_start(wout_sb, wout_d.rearrange("(po p) n -> p po n", p=128))

        qT_sb = qkv_pool.tile([128, NPAIR, N], bf16)   # [2 heads x 64d, pair, n]
        kT_sb = qkv_pool.tile([128, NPAIR, N], bf16)
        v_sb = qkv_pool.tile([128, NJT, CPC], bf16)    # [n row, j-tile, head*64+d]

        # ---- phase B: QKV projections (inside its own xT scope) ----
        with tc.tile_pool(name="xT", bufs=1) as xpool:
            xT_sb = xpool.tile([128, NKT, N], bf16)
            nc.sync.dma_start(xT_sb, xT_d.rearrange("(ko p) n -> p ko n", p=128))

            # qT / kT : out[c,i] = sum_k w[k,c] * xT[k,i]
            for p in range(NPAIR):
                for qk, dst in ((0, qT_sb), (1, kT_sb)):
                    woff = qk * CPC + p * 128
                    for ib in range(NIB):
                        ps = psum1s.tile([128, IB], f32, tag="qkvps")
                        for kt in range(NKT):
                            nc.tensor.matmul(
                                ps,
                                lhsT=wqkv_sb[:, kt, woff:woff + 128],
                                rhs=xT_sb[:, kt, ib * IB:(ib + 1) * IB],
                                start=(kt == 0), stop=(kt == NKT - 1))
                        nc.vector.tensor_copy(
                            out=dst[:, p, ib * IB:(ib + 1) * IB], in_=ps)
            # V : out[i,c] = sum_k xT[k,i] * w[k, 2*CPC + c]
            for jt in range(NJT):
                ps = psum1s.tile([128, CPC], f32, tag="qkvps")
                for kt in range(NKT):
                    nc.tensor.matmul(
                        ps,
                        lhsT=xT_sb[:, kt, jt * 128:(jt + 1) * 128],
                        rhs=wqkv_sb[:, kt, 2 * CPC:3 * CPC],
                        start=(kt == 0), stop=(kt == NKT - 1))
                nc.vector.tensor_copy(out=v_sb[:, jt, :], in_=ps)

        # ---- phase C: attention ----
        for ib in range(NIB):
            isl = slice(ib * IB, (ib + 1) * IB)
            oT_list = []
            for p in range(NPAIR):
                ptA = ppool.tile([128, NJT, IB], bf16, tag="ptA")
                ptB = ppool.tile([128, NJT, IB], bf16, tag="ptB")
                oT_ps = psum.tile([128, IB], f32, tag="oT")

                def emit_pv(q):
                    for t in range(2):
                        jt = 2 * q + t
                        st = (jt == 0)
                        sp = (jt == NJT - 1)
                        nc.tensor.matmul(
                            oT_ps[0:64, :],
                            lhsT=v_sb[:, jt, (2 * p) * DH:(2 * p + 1) * DH],
                            rhs=ptA[:, jt, :],
                            start=st, stop=sp, tile_position=(0, 0))
                        nc.tensor.matmul(
                            oT_ps[64:128, :],
                            lhsT=v_sb[:, jt, (2 * p + 1) * DH:(2 * p + 2) * DH],
                            rhs=ptB[:, jt, :],
                            start=st, stop=sp, tile_position=(0, 64))

                for q in range(NJT // 2):
                    sA = psum2.tile([128, 2 * IB], f32, tag="sA")
                    sB = psum.tile([128, 2 * IB], f32, tag="sB")
                    for t in range(2):
                        jt = 2 * q + t
                        jsl = slice(jt * 128, (jt + 1) * 128)
                        nc.tensor.matmul(
                            sA[:, t * IB:(t + 1) * IB],
                            lhsT=kT_sb[0:64, p, jsl],
                            rhs=qT_sb[0:64, p, isl],
                            start=True, stop=True, tile_position=(0, 0))
                        nc.tensor.matmul(
                            sB[:, t * IB:(t + 1) * IB],
                            lhsT=kT_sb[64:128, p, jsl],
                            rhs=qT_sb[64:128, p, isl],
                            start=True, stop=True, tile_position=(64, 0))
                    nc.scalar.activation(
                        ptA[:, 2 * q:2 * q + 2, :].rearrange("p a b -> p (a b)"),
                        sA, Exp, scale=SCALE)
                    nc.scalar.activation(
                        ptB[:, 2 * q:2 * q + 2, :].rearrange("p a b -> p (a b)"),
                        sB, Exp, scale=SCALE)
                    if q >= 1:
                        emit_pv(q - 1)
                emit_pv(NJT // 2 - 1)

                # denominators: sum P^T over all j (partitions x 16 tiles).
                # In-place add-tree inside the P^T tile (dead after PV reads).
                dns = []
                for pt in (ptA, ptB):
                    def half(lo, n):
                        return pt[:, lo:lo + n, :].rearrange("p a b -> p (a b)")
                    nc.vector.tensor_tensor(half(0, 8), half(0, 8), half(8, 8), add)
                    nc.vector.tensor_tensor(half(0, 4), half(0, 4), half(4, 4), add)
                    nc.vector.tensor_tensor(half(0, 2), half(0, 2), half(2, 2), add)
                    acc = spool.tile([128, IB], f32, tag="acc")
                    nc.vector.tensor_tensor(acc, half(0, 1), half(1, 1), add)
                    dn = spool.tile([128, IB], f32, tag="dn")
                    nc.gpsimd.partition_all_reduce(
                        dn, acc, channels=128, reduce_op=bass_isa.ReduceOp.add)
                    nc.vector.reciprocal(dn, dn)
                    dns.append(dn)

                oT_sb = opool.tile([128, IB], bf16, tag="oTsb")
                nc.vector.tensor_tensor(
                    oT_sb[0:64, :], oT_ps[0:64, :], dns[0][0:64, :], mult)
                nc.vector.tensor_tensor(
                    oT_sb[64:128, :], oT_ps[64:128, :], dns[1][64:128, :], mult)
                oT_list.append(oT_sb)

            # ---- output projection for this i-block ----
            for isub in range(4):
                for nh in range(2):
                    ops = psum1s.tile([128, 512], f32, tag="qkvps")
                    for p in range(NPAIR):
                        nc.tensor.matmul(
                            ops,
                            lhsT=oT_list[p][:, isub * 128:(isub + 1) * 128],
                            rhs=wout_sb[:, p, nh * 512:(nh + 1) * 512],
                            start=(p == 0), stop=(p == NPAIR - 1))
                    ost = outp.tile([128, 512], f32, tag="ost")
                    nc.vector.tensor_copy(out=ost, in_=ops)
                    nc.sync.dma_start(
                        out_d[ib * IB + isub * 128: ib * IB + (isub + 1) * 128,
                              nh * 512:(nh + 1) * 512], ost)


def _prep_inputs(x, w_qkv, w_out):
    bf = ml_dtypes.bfloat16
    in_maps = []
    for c in range(8):
        b, hh = c // 2, c % 2
        xT = np.ascontiguousarray(x[b].T).astype(bf)
        q = w_qkv[:, hh * CPC:(hh + 1) * CPC]
        k = w_qkv[:, DIM + hh * CPC: DIM + (hh + 1) * CPC]
        v = w_qkv[:, 2 * DIM + hh * CPC: 2 * DIM + (hh + 1) * CPC]
        wqkv = np.ascontiguousarray(np.concatenate([q, k, v], axis=1)).astype(bf)
        wout = np.ascontiguousarray(w_out[hh * CPC:(hh + 1) * CPC, :]).astype(bf)
        in_maps.append({"xT": xT, "wqkv": wqkv, "wout": wout})
    return in_maps


def _run(x, w_qkv, w_out, b_out, trace=False):
    from concourse import bass_utils
    if "nc" not in _cache:
        _cache["nc"] = _build()
    nc = _cache["nc"]
    in_maps = _prep_inputs(x, w_qkv, w_out)
    res = bass_utils.run_bass_kernel_spmd(
        nc, in_maps, core_ids=list(range(8)), trace=trace)
    partials = [r["out"] for r in res.results]
    out = np.empty((B, N, DIM), dtype=np.float32)
    for b in range(B):
        out[b] = partials[2 * b] + partials[2 * b + 1] + b_out.astype(np.float32)
    return out, res


def kernel(x, w_qkv, w_out, b_out):
    x = np.asarray(x, dtype=np.float32)
    w_qkv = np.asarray(w_qkv, dtype=np.float32)
    w_out = np.asarray(w_out, dtype=np.float32)
    b_out = np.asarray(b_out, dtype=np.float32)
    out, _ = _run(x, w_qkv, w_out, b_out, trace=False)
    return out


# revision 9
# speedup vs baseline: 1.2360x; 1.2360x over previous
"""Multi-head attention (B=4, N=2048, DIM=1024, H=16, DH=64) on 8 trn2 cores.

Sharding: core c handles batch c//2 and head-half c%2 (8 heads).  Each core
computes qkv projection for its heads, attention, and a partial output
projection; the host sums the two partials per batch and adds the bias.
No cross-core collectives needed.

Layout strategy (zero on-device transposes):
  - host supplies x[b] pre-transposed (xT: [DIM, N]) in bf16
  - qT/kT computed as [d, n] ("transposed") via out = W^T @ x^T matmuls
  - S^T tiles [j=128, i=512] from row-packed matmuls (d=64 contraction,
    2 heads concurrently in PE row groups 0-63 / 64-127)
  - exp via ACT (scale folded), PSUM -> SBUF bf16 (P^T tiles)
  - PV: O^T[d, i] += V[j, d]^T-matmul, col-packed pairs (PE col groups)
  - denominators: DVE add-tree over j-tiles + gpsimd partition_all_reduce
  - output projection consumes O^T tiles directly as lhsT
"""

import numpy as np
import ml_dtypes

B, N, DIM = 4, 2048, 1024
HEADS, DH = 16, 64
SCALE = DIM ** (-0.5)
HPC = 8              # heads per core
NPAIR = HPC // 2     # 4 head pairs
CPC = HPC * DH       # 512 channels per core
IB = 512             # i-block (query cols per attention unit)
NIB = N // IB        # 4
NJT = N // 128       # 16 j-tiles
NKT = DIM // 128     # 8 contraction tiles for projections

_cache = {}


def _build():
    import concourse.bacc as bacc
    import concourse.mybir as mybir
    import concourse.tile as tile

    f32 = mybir.dt.float32
    bf16 = mybir.dt.bfloat16

    nc = bacc.Bacc("TRN2", target_bir_lowering=False, debug=False,
                   enable_asserts=False, num_devices=8)

    xT_d = nc.dram_tensor("xT", (DIM, N), bf16, kind="ExternalInput").ap()
    wqkv_d = nc.dram_tensor("wqkv", (DIM, 3 * CPC), bf16, kind="ExternalInput").ap()
    wout_d = nc.dram_tensor("wout", (CPC, DIM), bf16, kind="ExternalInput").ap()
    out_d = nc.dram_tensor("out", (N, DIM), f32, kind="ExternalOutput").ap()

    with tile.TileContext(nc) as tc:
        _body(nc, tc, mybir, xT_d, wqkv_d, wout_d, out_d)

    nc.compile()
    return nc


def _body(nc, tc, mybir, xT_d, wqkv_d, wout_d, out_d):
    import concourse.bass_isa as bass_isa
    from contextlib import ExitStack

    f32 = mybir.dt.float32
    bf16 = mybir.dt.bfloat16
    Exp = mybir.ActivationFunctionType.Exp
    mult = mybir.AluOpType.mult
    add = mybir.AluOpType.add

    ctx = ExitStack()
    with ctx:
        wpool = ctx.enter_context(tc.tile_pool(name="weights", bufs=1))
        qkv_pool = ctx.enter_context(tc.tile_pool(name="qkv", bufs=1))
        ppool = ctx.enter_context(tc.tile_pool(name="ptiles", bufs=2))
        spool = ctx.enter_context(tc.tile_pool(name="small", bufs=2))
        opool = ctx.enter_context(tc.tile_pool(name="oT", bufs=8))
        outp = ctx.enter_context(tc.tile_pool(name="outstage", bufs=3))
        psum = ctx.enter_context(tc.tile_pool(name="psum", bufs=2, space="PSUM"))

        # ---- load weights ----
        wqkv_sb = wpool.tile([128, NKT, 3 * CPC], bf16)
        wqkv_r = wqkv_d.rearrange("(ko p) c -> p ko c", p=128)
        for kt in range(NKT):
            nc.sync.dma_start(wqkv_sb[:, kt, :], wqkv_r[:, kt, :])
        wout_sb = wpool.tile([128, NPAIR, DIM], bf16)
        nc.sync.dma_start(wout_sb, wout_d.rearrange("(po p) n -> p po n", p=128))

        qT_sb = qkv_pool.tile([128, NPAIR, N], bf16)   # [2 heads x 64d, pair, n]
        kT_sb = qkv_pool.tile([128, NPAIR, N], bf16)
        v_sb = qkv_pool.tile([128, NJT, CPC], bf16)    # [n row, j-tile, head*64+d]

        # ---- phase B: QKV projections (inside its own xT scope) ----
        with tc.tile_pool(name="xT", bufs=1) as xpool:
            xT_sb = xpool.tile([128, NKT, N], bf16)
            xT_r = xT_d.rearrange("(ko p) n -> p ko n", p=128)
            for kt in range(NKT):
                nc.sync.dma_start(xT_sb[:, kt, :], xT_r[:, kt, :])

            # qT / kT : out[c,i] = sum_k w[k,c] * xT[k,i]
            for p in range(NPAIR):
                for qk, dst in ((0, qT_sb), (1, kT_sb)):
                    woff = qk * CPC + p * 128
                    for ib in range(NIB):
                        ps = psum.tile([128, IB], f32, tag="qkvps")
                        for kt in range(NKT):
                            nc.tensor.matmul(
                                ps,
                                lhsT=wqkv_sb[:, kt, woff:woff + 128],
                                rhs=xT_sb[:, kt, ib * IB:(ib + 1) * IB],
                                start=(kt == 0), stop=(kt == NKT - 1))
                        nc.vector.tensor_copy(
                            out=dst[:, p, ib * IB:(ib + 1) * IB], in_=ps)
            # V : out[i,c] = sum_k xT[k,i] * w[k, 2*CPC + c]
            for jt in range(NJT):
                ps = psum.tile([128, CPC], f32, tag="qkvps")
                for kt in range(NKT):
                    nc.tensor.matmul(
                        ps,
                        lhsT=xT_sb[:, kt, jt * 128:(jt + 1) * 128],
                        rhs=wqkv_sb[:, kt, 2 * CPC:3 * CPC],
                        start=(kt == 0), stop=(kt == NKT - 1))
                nc.vector.tensor_copy(out=v_sb[:, jt, :], in_=ps)

        # ---- phase C: attention ----
        # Per (pair, i-block) unit: for each j-tile, one [128, 1024] PSUM
        # tile holds S^T for both heads of the pair (A in cols 0:512, B in
        # 512:1024) -> ONE exp instruction per j-tile.  P^T lives in a
        # single [128, 2*NJT, IB] tile with A/B interleaved (index 2jt+h).
        for ib in range(NIB):
            isl = slice(ib * IB, (ib + 1) * IB)
            oT_list = []
            for p in range(NPAIR):
                pt = ppool.tile([128, 2 * NJT, IB], bf16, tag="pt")
                oT_ps = psum.tile([128, IB], f32, tag="oT")

                def emit_pv(jt):
                    st = (jt == 0)
                    sp = (jt == NJT - 1)
                    nc.tensor.matmul(
                        oT_ps[0:64, :],
                        lhsT=v_sb[:, jt, (2 * p) * DH:(2 * p + 1) * DH],
                        rhs=pt[:, 2 * jt, :],
                        start=st, stop=sp, tile_position=(0, 0))
                    nc.tensor.matmul(
                        oT_ps[64:128, :],
                        lhsT=v_sb[:, jt, (2 * p + 1) * DH:(2 * p + 2) * DH],
                        rhs=pt[:, 2 * jt + 1, :],
                        start=st, stop=sp, tile_position=(0, 64))

                for jt in range(NJT):
                    jsl = slice(jt * 128, (jt + 1) * 128)
                    sAB = psum.tile([128, 2 * IB], f32, tag="sAB")
                    nc.tensor.matmul(
                        sAB[:, 0:IB],
                        lhsT=kT_sb[0:64, p, jsl],
                        rhs=qT_sb[0:64, p, isl],
                        start=True, stop=True, tile_position=(0, 0))
                    nc.tensor.matmul(
                        sAB[:, IB:2 * IB],
                        lhsT=kT_sb[64:128, p, jsl],
                        rhs=qT_sb[64:128, p, isl],
                        start=True, stop=True, tile_position=(64, 0))
                    nc.scalar.activation(
                        pt[:, 2 * jt:2 * jt + 2, :].rearrange("p a b -> p (a b)"),
                        sAB, Exp, scale=SCALE)
                    if jt >= 1:
                        emit_pv(jt - 1)
                emit_pv(NJT - 1)

                # denominators for both heads at once: in-place add-tree over
                # j inside pt (dead after PV reads), then gpsimd all-reduce
                # over partitions and a fast reciprocal.
                def half(lo, n):
                    return pt[:, lo:lo + n, :].rearrange("p a b -> p (a b)")
                nc.vector.tensor_tensor(half(0, 16), half(0, 16), half(16, 16), add)
                nc.vector.tensor_tensor(half(0, 8), half(0, 8), half(8, 8), add)
                nc.vector.tensor_tensor(half(0, 4), half(0, 4), half(4, 4), add)
                acc = spool.tile([128, 2, IB], f32, tag="acc")
                nc.vector.tensor_tensor(
                    acc.rearrange("p a b -> p (a b)"), half(0, 2), half(2, 2), add)
                dn = spool.tile([128, 2, IB], f32, tag="dn")
                nc.gpsimd.partition_all_reduce(
                    dn.rearrange("p a b -> p (a b)"),
                    acc.rearrange("p a b -> p (a b)"),
                    channels=128, reduce_op=bass_isa.ReduceOp.add)
                nc.vector.reciprocal_approx_fast(
                    dn.rearrange("p a b -> p (a b)"),
                    dn.rearrange("p a b -> p (a b)"))

                oT_sb = opool.tile([128, IB], bf16, tag="oTsb")
                nc.vector.tensor_tensor(
                    oT_sb[0:64, :], oT_ps[0:64, :], dn[0:64, 0, :], mult)
                nc.vector.tensor_tensor(
                    oT_sb[64:128, :], oT_ps[64:128, :], dn[64:128, 1, :], mult)
                oT_list.append(oT_sb)

            # ---- output projection for this i-block ----
            for isub in range(4):
                for nh in range(2):
                    ops = psum.tile([128, 512], f32, tag="qkvps")
                    for p in range(NPAIR):
                        nc.tensor.matmul(
                            ops,
                            lhsT=oT_list[p][:, isub * 128:(isub + 1) * 128],
                            rhs=wout_sb[:, p, nh * 512:(nh + 1) * 512],
                            start=(p == 0), stop=(p == NPAIR - 1))
                    ost = outp.tile([128, 512], f32, tag="ost")
                    nc.vector.tensor_copy(out=ost, in_=ops)
                    nc.sync.dma_start(
                        out_d[ib * IB + isub * 128: ib * IB + (isub + 1) * 128,
                              nh * 512:(nh + 1) * 512], ost)


def _prep_inputs(x, w_qkv, w_out):
    bf = ml_dtypes.bfloat16
    in_maps = []
    for c in range(8):
        b, hh = c // 2, c % 2
        xT = np.ascontiguousarray(x[b].T).astype(bf)
        q = w_qkv[:, hh * CPC:(hh + 1) * CPC]
        k = w_qkv[:, DIM + hh * CPC: DIM + (hh + 1) * CPC]
        v = w_qkv[:, 2 * DIM + hh * CPC: 2 * DIM + (hh + 1) * CPC]
        wqkv = np.ascontiguousarray(np.concatenate([q, k, v], axis=1)).astype(bf)
        wout = np.ascontiguousarray(w_out[hh * CPC:(hh + 1) * CPC, :]).astype(bf)
        in_maps.append({"xT": xT, "wqkv": wqkv, "wout": wout})
    return in_maps


def _run(x, w_qkv, w_out, b_out, trace=False):
    from concourse import bass_utils
    if "nc" not in _cache:
        _cache["nc"] = _build()
    nc = _cache["nc"]
    in_maps = _prep_inputs(x, w_qkv, w_out)
    res = bass_utils.run_bass_kernel_spmd(
        nc, in_maps, core_ids=list(range(8)), trace=trace)
    partials = [r["out"] for r in res.results]
    out = np.empty((B, N, DIM), dtype=np.float32)
    for b in range(B):
        out[b] = partials[2 * b] + partials[2 * b + 1] + b_out.astype(np.float32)
    return out, res


def kernel(x, w_qkv, w_out, b_out):
    x = np.asarray(x, dtype=np.float32)
    w_qkv = np.asarray(w_qkv, dtype=np.float32)
    w_out = np.asarray(w_out, dtype=np.float32)
    b_out = np.asarray(b_out, dtype=np.float32)
    out, _ = _run(x, w_qkv, w_out, b_out, trace=False)
    return out


# revision 11
# speedup vs baseline: 1.3483x; 1.0909x over previous
"""Multi-head attention (B=4, N=2048, DIM=1024, H=16, DH=64) on 8 trn2 cores.

Sharding: core c handles batch c//2 and head-half c%2 (8 heads).  Each core
computes qkv projection for its heads, attention, and a partial output
projection; the host sums the two partials per batch and adds the bias.
No cross-core collectives needed.

Layout strategy (zero on-device transposes):
  - host supplies x[b] pre-transposed (xT: [DIM, N]) in bf16
  - qT/kT computed as [d, n] ("transposed") via out = W^T @ x^T matmuls
  - S^T tiles [j=128, i=512] from row-packed matmuls (d=64 contraction,
    2 heads concurrently in PE row groups 0-63 / 64-127)
  - exp via ACT (scale folded), PSUM -> SBUF bf16 (P^T tiles)
  - PV: O^T[d, i] += V[j, d]^T-matmul, col-packed pairs (PE col groups)
  - denominators: DVE add-tree over j-tiles + gpsimd partition_all_reduce
  - output projection consumes O^T tiles directly as lhsT
"""

import numpy as np
import ml_dtypes

B, N, DIM = 4, 2048, 1024
HEADS, DH = 16, 64
SCALE = DIM ** (-0.5)
HPC = 8              # heads per core
NPAIR = HPC // 2     # 4 head pairs
CPC = HPC * DH       # 512 channels per core
IB = 512             # i-block (query cols per attention unit)
NIB = N // IB        # 4
NJT = N // 128       # 16 j-tiles
NKT = DIM // 128     # 8 contraction tiles for projections

_cache = {}


def _build():
    import concourse.bacc as bacc
    import concourse.mybir as mybir
    import concourse.tile as tile

    f32 = mybir.dt.float32
    bf16 = mybir.dt.bfloat16

    nc = bacc.Bacc("TRN2", target_bir_lowering=False, debug=False,
                   enable_asserts=False, num_devices=8)

    xT_d = nc.dram_tensor("xT", (DIM, N), bf16, kind="ExternalInput").ap()
    wqkv_d = nc.dram_tensor("wqkv", (DIM, 3 * CPC), bf16, kind="ExternalInput").ap()
    wout_d = nc.dram_tensor("wout", (CPC, DIM), bf16, kind="ExternalInput").ap()
    out_d = nc.dram_tensor("out", (N, DIM), f32, kind="ExternalOutput").ap()

    with tile.TileContext(nc) as tc:
        _body(nc, tc, mybir, xT_d, wqkv_d, wout_d, out_d)

    nc.compile()
    return nc


def _body(nc, tc, mybir, xT_d, wqkv_d, wout_d, out_d):
    import concourse.bass_isa as bass_isa
    from contextlib import ExitStack

    f32 = mybir.dt.float32
    bf16 = mybir.dt.bfloat16
    Exp = mybir.ActivationFunctionType.Exp
    mult = mybir.AluOpType.mult
    add = mybir.AluOpType.add

    ctx = ExitStack()
    with ctx:
        wpool = ctx.enter_context(tc.tile_pool(name="weights", bufs=1))
        qkv_pool = ctx.enter_context(tc.tile_pool(name="qkv", bufs=1))
        ppool = ctx.enter_context(tc.tile_pool(name="ptiles", bufs=2))
        spool = ctx.enter_context(tc.tile_pool(name="small", bufs=2))
        outp = ctx.enter_context(tc.tile_pool(name="outstage", bufs=3))
        psum = ctx.enter_context(tc.tile_pool(name="psum", bufs=2, space="PSUM"))

        # ---- load weights ----
        wqkv_sb = wpool.tile([128, NKT, 3 * CPC], bf16)
        wqkv_r = wqkv_d.rearrange("(ko p) c -> p ko c", p=128)
        for kt in range(NKT):
            nc.sync.dma_start(wqkv_sb[:, kt, :], wqkv_r[:, kt, :])
        wout_sb = wpool.tile([128, NPAIR, DIM], bf16)
        nc.sync.dma_start(wout_sb, wout_d.rearrange("(po p) n -> p po n", p=128))

        qT_sb = qkv_pool.tile([128, NPAIR, N], bf16)   # [2 heads x 64d, pair, n]
        kT_sb = qkv_pool.tile([128, NPAIR, N], bf16)
        v_sb = qkv_pool.tile([128, NJT, CPC], bf16)    # [n row, j-tile, head*64+d]

        # ---- phase B: QKV projections (inside its own xT scope) ----
        with tc.tile_pool(name="xT", bufs=1) as xpool:
            xT_sb = xpool.tile([128, NKT, N], bf16)
            xT_r = xT_d.rearrange("(ko p) n -> p ko n", p=128)
            for kt in range(NKT):
                nc.sync.dma_start(xT_sb[:, kt, :], xT_r[:, kt, :])

            # qT / kT : out[c,i] = sum_k w[k,c] * xT[k,i]
            for p in range(NPAIR):
                for qk, dst in ((0, qT_sb), (1, kT_sb)):
                    woff = qk * CPC + p * 128
                    for ib in range(NIB):
                        ps = psum.tile([128, IB], f32, tag="qkvps")
                        for kt in range(NKT):
                            nc.tensor.matmul(
                                ps,
                                lhsT=wqkv_sb[:, kt, woff:woff + 128],
                                rhs=xT_sb[:, kt, ib * IB:(ib + 1) * IB],
                                start=(kt == 0), stop=(kt == NKT - 1))
                        nc.vector.tensor_copy(
                            out=dst[:, p, ib * IB:(ib + 1) * IB], in_=ps)
            # V : out[i,c] = sum_k xT[k,i] * w[k, 2*CPC + c]
            for jt in range(NJT):
                ps = psum.tile([128, CPC], f32, tag="qkvps")
                for kt in range(NKT):
                    nc.tensor.matmul(
                        ps,
                        lhsT=xT_sb[:, kt, jt * 128:(jt + 1) * 128],
                        rhs=wqkv_sb[:, kt, 2 * CPC:3 * CPC],
                        start=(kt == 0), stop=(kt == NKT - 1))
                nc.vector.tensor_copy(out=v_sb[:, jt, :], in_=ps)

        # ---- phase C: attention ----
        # Per (pair, i-block) unit: for each j-tile, one [128, 1024] PSUM
        # tile holds S^T for both heads of the pair (A in cols 0:512, B in
        # 512:1024) -> ONE exp instruction per j-tile.  P^T lives in a
        # single [128, 2*NJT, IB] tile with A/B interleaved (index 2jt+h).
        # Loop order: pair-outer / i-block-inner; the output projection for
        # i-block ib is interleaved into the j-loop of unit (p3, ib+1) to
        # keep the tensor engine dense (HAM-warm).
        opool = ctx.enter_context(tc.tile_pool(name="oT", bufs=18))
        oT_all = {}

        def outproj_steps(ib):
            steps = []
            for isub in range(4):
                for nh in range(2):
                    def step(isub=isub, nh=nh):
                        ops = psum.tile([128, 512], f32, tag="qkvps")
                        for p in range(NPAIR):
                            nc.tensor.matmul(
                                ops,
                                lhsT=oT_all[(p, ib)][:, isub * 128:(isub + 1) * 128],
                                rhs=wout_sb[:, p, nh * 512:(nh + 1) * 512],
                                start=(p == 0), stop=(p == NPAIR - 1))
                        ost = outp.tile([128, 512], f32, tag="ost")
                        nc.vector.tensor_copy(out=ost, in_=ops)
                        nc.sync.dma_start(
                            out_d[ib * IB + isub * 128: ib * IB + (isub + 1) * 128,
                                  nh * 512:(nh + 1) * 512], ost)
                    steps.append(step)
            return steps

        def unit(p, ib, extra):
            isl = slice(ib * IB, (ib + 1) * IB)
            pt = ppool.tile([128, 2 * NJT, IB], bf16, tag="pt")
            oT_ps = psum.tile([128, IB], f32, tag="oT")

            def emit_pv(jt):
                st = (jt == 0)
                sp = (jt == NJT - 1)
                nc.tensor.matmul(
                    oT_ps[0:64, :],
                    lhsT=v_sb[:, jt, (2 * p) * DH:(2 * p + 1) * DH],
                    rhs=pt[:, 2 * jt, :],
                    start=st, stop=sp, tile_position=(0, 0))
                nc.tensor.matmul(
                    oT_ps[64:128, :],
                    lhsT=v_sb[:, jt, (2 * p + 1) * DH:(2 * p + 2) * DH],
                    rhs=pt[:, 2 * jt + 1, :],
                    start=st, stop=sp, tile_position=(0, 64))

            for jt in range(NJT):
                jsl = slice(jt * 128, (jt + 1) * 128)
                sAB = psum.tile([128, 2 * IB], f32, tag="sAB")
                nc.tensor.matmul(
                    sAB[:, 0:IB],
                    lhsT=kT_sb[0:64, p, jsl],
                    rhs=qT_sb[0:64, p, isl],
                    start=True, stop=True, tile_position=(0, 0))
                nc.tensor.matmul(
                    sAB[:, IB:2 * IB],
                    lhsT=kT_sb[64:128, p, jsl],
                    rhs=qT_sb[64:128, p, isl],
                    start=True, stop=True, tile_position=(64, 0))
                nc.scalar.activation(
                    pt[:, 2 * jt:2 * jt + 2, :].rearrange("p a b -> p (a b)"),
                    sAB, Exp, scale=SCALE)
                if jt >= 1:
                    emit_pv(jt - 1)
                if jt % 2 == 0 and jt // 2 < len(extra):
                    extra[jt // 2]()
            emit_pv(NJT - 1)

            # denominators for both heads at once: in-place add-tree over
            # j inside pt (dead after PV reads), then gpsimd all-reduce
            # over partitions and a fast reciprocal.
            def half(lo, n):
                return pt[:, lo:lo + n, :].rearrange("p a b -> p (a b)")
            nc.vector.tensor_tensor(half(0, 16), half(0, 16), half(16, 16), add)
            nc.vector.tensor_tensor(half(0, 8), half(0, 8), half(8, 8), add)
            nc.vector.tensor_tensor(half(0, 4), half(0, 4), half(4, 4), add)
            acc = spool.tile([128, 2, IB], f32, tag="acc")
            nc.vector.tensor_tensor(
                acc.rearrange("p a b -> p (a b)"), half(0, 2), half(2, 2), add)
            dn = spool.tile([128, 2, IB], f32, tag="dn")
            nc.gpsimd.partition_all_reduce(
                dn.rearrange("p a b -> p (a b)"),
                acc.rearrange("p a b -> p (a b)"),
                channels=128, reduce_op=bass_isa.ReduceOp.add)
            nc.vector.reciprocal_approx_fast(
                dn.rearrange("p a b -> p (a b)"),
                dn.rearrange("p a b -> p (a b)"))

            oT_sb = opool.tile([128, IB], bf16, tag="oTsb")
            nc.vector.tensor_tensor(
                oT_sb[0:64, :], oT_ps[0:64, :], dn[0:64, 0, :], mult)
            nc.vector.tensor_tensor(
                oT_sb[64:128, :], oT_ps[64:128, :], dn[64:128, 1, :], mult)
            oT_all[(p, ib)] = oT_sb

        for p in range(NPAIR):
            for ib in range(NIB):
                extra = []
                if p == NPAIR - 1 and ib >= 1:
                    extra = outproj_steps(ib - 1)
                unit(p, ib, extra)
        for step in outproj_steps(NIB - 1):
            step()


def _prep_inputs(x, w_qkv, w_out):
    bf = ml_dtypes.bfloat16
    in_maps = []
    for c in range(8):
        b, hh = c // 2, c % 2
        xT = np.ascontiguousarray(x[b].T).astype(bf)
        q = w_qkv[:, hh * CPC:(hh + 1) * CPC]
        k = w_qkv[:, DIM + hh * CPC: DIM + (hh + 1) * CPC]
        v = w_qkv[:, 2 * DIM + hh * CPC: 2 * DIM + (hh + 1) * CPC]
        wqkv = np.ascontiguousarray(np.concatenate([q, k, v], axis=1)).astype(bf)
        wout = np.ascontiguousarray(w_out[hh * CPC:(hh + 1) * CPC, :]).astype(bf)
        in_maps.append({"xT": xT, "wqkv": wqkv, "wout": wout})
    return in_maps


def _run(x, w_qkv, w_out, b_out, trace=False):
    from concourse import bass_utils
    if "nc" not in _cache:
        _cache["nc"] = _build()
    nc = _cache["nc"]
    in_maps = _prep_inputs(x, w_qkv, w_out)
    res = bass_utils.run_bass_kernel_spmd(
        nc, in_maps, core_ids=list(range(8)), trace=trace)
    partials = [r["out"] for r in res.results]
    out = np.empty((B, N, DIM), dtype=np.float32)
    for b in range(B):
        out[b] = partials[2 * b] + partials[2 * b + 1] + b_out.astype(np.float32)
    return out, res


def kernel(x, w_qkv, w_out, b_out):
    x = np.asarray(x, dtype=np.float32)
    w_qkv = np.asarray(w_qkv, dtype=np.float32)
    w_out = np.asarray(w_out, dtype=np.float32)
    b_out = np.asarray(b_out, dtype=np.float32)
    out, _ = _run(x, w_qkv, w_out, b_out, trace=False)
    return out


# revision 16
# speedup vs baseline: 1.4761x; 1.0948x over previous
"""Multi-head attention (B=4, N=2048, DIM=1024, H=16, DH=64) on 8 trn2 cores.

Sharding: core c handles batch c//2 and head-half c%2 (8 heads).  Each core
computes qkv projection for its heads, attention, and a partial output
projection; the host sums the two partials per batch and adds the bias.
No cross-core collectives needed.

Layout strategy (zero on-device transposes):
  - host supplies x[b] pre-transposed (xT: [DIM, N]) in bf16
  - qT/kT computed as [d, n] ("transposed") via out = W^T @ x^T matmuls
  - S^T tiles [j=128, i=512] from row-packed matmuls (d=64 contraction,
    2 heads concurrently in PE row groups 0-63 / 64-127)
  - exp via ACT (scale folded), PSUM -> SBUF bf16 (P^T tiles)
  - PV: O^T[d, i] += V[j, d]^T-matmul, col-packed pairs (PE col groups)
  - denominators: DVE add-tree over j-tiles + gpsimd partition_all_reduce
  - output projection consumes O^T tiles directly as lhsT
"""

import numpy as np
import ml_dtypes

B, N, DIM = 4, 2048, 1024
HEADS, DH = 16, 64
SCALE = DIM ** (-0.5)
HPC = 8              # heads per core
NPAIR = HPC // 2     # 4 head pairs
CPC = HPC * DH       # 512 channels per core
IB = 512             # i-block (query cols per attention unit)
NIB = N // IB        # 4
NJT = N // 128       # 16 j-tiles
NKT = DIM // 128     # 8 contraction tiles for projections

_cache = {}


def _build():
    import concourse.bacc as bacc
    import concourse.mybir as mybir
    import concourse.tile as tile

    f32 = mybir.dt.float32
    bf16 = mybir.dt.bfloat16

    nc = bacc.Bacc("TRN2", target_bir_lowering=False, debug=False,
                   enable_asserts=False, num_devices=8)

    xT_d = nc.dram_tensor("xT", (DIM, N), bf16, kind="ExternalInput").ap()
    wqkv_d = nc.dram_tensor("wqkv", (DIM, 3 * CPC), bf16, kind="ExternalInput").ap()
    wout_d = nc.dram_tensor("wout", (CPC, DIM), bf16, kind="ExternalInput").ap()
    out_d = nc.dram_tensor("out", (N, DIM), f32, kind="ExternalOutput").ap()

    with tile.TileContext(nc) as tc:
        _body(nc, tc, mybir, xT_d, wqkv_d, wout_d, out_d)

    nc.compile()
    return nc


def _body(nc, tc, mybir, xT_d, wqkv_d, wout_d, out_d):
    import concourse.bass_isa as bass_isa
    from contextlib import ExitStack

    f32 = mybir.dt.float32
    bf16 = mybir.dt.bfloat16
    Exp = mybir.ActivationFunctionType.Exp
    mult = mybir.AluOpType.mult
    add = mybir.AluOpType.add
    NJH = NJT // 2   # j-tiles per half (8)

    ctx = ExitStack()
    with ctx:
        wpool = ctx.enter_context(tc.tile_pool(name="weights", bufs=1))
        qkv_pool = ctx.enter_context(tc.tile_pool(name="qkv", bufs=1))
        ppool = ctx.enter_context(tc.tile_pool(name="ptiles", bufs=2))
        ppool1 = ctx.enter_context(tc.tile_pool(name="ptiles1", bufs=1))
        spool = ctx.enter_context(tc.tile_pool(name="small", bufs=2))
        outp = ctx.enter_context(tc.tile_pool(name="outstage", bufs=3))
        opool = ctx.enter_context(tc.tile_pool(name="oT", bufs=16))
        psum = ctx.enter_context(tc.tile_pool(name="psum", bufs=2, space="PSUM"))

        # ---- weights + xT load (per k-tile so compute starts early) ----
        wqkv_sb = wpool.tile([128, NKT, 3 * CPC], bf16)
        wqkv_r = wqkv_d.rearrange("(ko p) c -> p ko c", p=128)
        for kt in range(NKT):
            nc.sync.dma_start(wqkv_sb[:, kt, :], wqkv_r[:, kt, :])
        wout_sb = wpool.tile([128, NPAIR, DIM], bf16)
        nc.sync.dma_start(wout_sb, wout_d.rearrange("(po p) n -> p po n", p=128))
        xT_sb = wpool.tile([128, NKT, N], bf16)
        xT_r = xT_d.rearrange("(ko p) n -> p ko n", p=128)
        for kt in range(NKT):
            nc.sync.dma_start(xT_sb[:, kt, :], xT_r[:, kt, :])

        # per-pair q/k tiles (separate tiles => clean dependency tracking
        # when later pairs' projections interleave into attention units)
        qT_t = [qkv_pool.tile([128, N], bf16, tag=f"qT{p}", name=f"qT{p}") for p in range(NPAIR)]
        kT_t = [qkv_pool.tile([128, N], bf16, tag=f"kT{p}", name=f"kT{p}") for p in range(NPAIR)]
        v_sb = qkv_pool.tile([128, NJT, CPC], bf16)

        # ---- emit helpers ----
        def qk_steps(p):
            """Projection of qT/kT for pair p as a list of small PE bursts."""
            steps = []
            for qk, dst in ((0, qT_t[p]), (1, kT_t[p])):
                woff = qk * CPC + p * 128
                for ib in range(NIB):
                    cell = {}

                    def stepA(cell=cell, woff=woff, ib=ib):
                        cell["ps"] = psum.tile([128, IB], f32, tag="qkvps", name="qkps")
                        for kt in range(4):
                            nc.tensor.matmul(
                                cell["ps"],
                                lhsT=wqkv_sb[:, kt, woff:woff + 128],
                                rhs=xT_sb[:, kt, ib * IB:(ib + 1) * IB],
                                start=(kt == 0), stop=False)

                    def stepB(cell=cell, woff=woff, ib=ib, dst=dst):
                        for kt in range(4, NKT):
                            nc.tensor.matmul(
                                cell["ps"],
                                lhsT=wqkv_sb[:, kt, woff:woff + 128],
                                rhs=xT_sb[:, kt, ib * IB:(ib + 1) * IB],
                                start=False, stop=(kt == NKT - 1))
                        nc.vector.tensor_copy(
                            out=dst[:, ib * IB:(ib + 1) * IB], in_=cell["ps"])

                    steps += [stepA, stepB]
            return steps

        def emit_v(jt):
            ps = psum.tile([128, CPC], f32, tag="qkvps")
            for kt in range(NKT):
                nc.tensor.matmul(
                    ps,
                    lhsT=xT_sb[:, kt, jt * 128:(jt + 1) * 128],
                    rhs=wqkv_sb[:, kt, 2 * CPC:3 * CPC],
                    start=(kt == 0), stop=(kt == NKT - 1))
            nc.vector.tensor_copy(out=v_sb[:, jt, :], in_=ps)

        oT_all = {}

        def outproj_steps(ib):
            steps = []
            for isub in range(4):
                for nh in range(2):
                    def step(isub=isub, nh=nh, ib=ib):
                        ops = psum.tile([128, 512], f32, tag="qkvps")
                        for p in range(NPAIR):
                            nc.tensor.matmul(
                                ops,
                                lhsT=oT_all[(p, ib)][:, isub * 128:(isub + 1) * 128],
                                rhs=wout_sb[:, p, nh * 512:(nh + 1) * 512],
                                start=(p == 0), stop=(p == NPAIR - 1))
                        ost = outp.tile([128, 512], f32, tag="ost")
                        nc.vector.tensor_copy(out=ost, in_=ops)
                        nc.sync.dma_start(
                            out_d[ib * IB + isub * 128: ib * IB + (isub + 1) * 128,
                                  nh * 512:(nh + 1) * 512], ost)
                    steps.append(step)
            return steps

        # ---- attention unit ----
        # P^T for a unit lives in two half tiles (j-tiles 0-7 / 8-15), each
        # [128, 2*NJH, IB] bf16 with planes indexed 2*jt_local + head.
        # Denominator add-tree runs in-place per half; the lo-half tree is
        # emitted inside the j-loop once PV consumed it.  The PAR-dependent
        # tail (reciprocal + normalize) is returned as a closure and
        # injected into the NEXT unit's j-loop (hides gpsimd latency from
        # the in-order DVE queue).
        def unit(p, ib, extras):
            isl = slice(ib * IB, (ib + 1) * IB)
            lo = ppool.tile([128, 2 * NJH, IB], bf16, tag="ptlo")
            hi = ppool1.tile([128, 2 * NJH, IB], bf16, tag="pthi")
            oT_ps = psum.tile([128, IB], f32, tag="oT")

            def pthalf(jt):
                return (lo, jt) if jt < NJH else (hi, jt - NJH)

            def emit_pv(jt):
                t, j = pthalf(jt)
                st = (jt == 0)
                sp = (jt == NJT - 1)
                nc.tensor.matmul(
                    oT_ps[0:64, :],
                    lhsT=v_sb[:, jt, (2 * p) * DH:(2 * p + 1) * DH],
                    rhs=t[:, 2 * j, :],
                    start=st, stop=sp, tile_position=(0, 0))
                nc.tensor.matmul(
                    oT_ps[64:128, :],
                    lhsT=v_sb[:, jt, (2 * p + 1) * DH:(2 * p + 2) * DH],
                    rhs=t[:, 2 * j + 1, :],
                    start=st, stop=sp, tile_position=(0, 64))

            def tree(t, n_ops=3):
                def half(lo_, n):
                    return t[:, lo_:lo_ + n, :].rearrange("p a b -> p (a b)")
                ops = []
                ops.append(lambda: nc.vector.tensor_tensor(
                    half(0, 8), half(0, 8), half(8, 8), add))
                ops.append(lambda: nc.vector.tensor_tensor(
                    half(0, 4), half(0, 4), half(4, 4), add))
                ops.append(lambda: nc.vector.tensor_tensor(
                    half(0, 2), half(0, 2), half(2, 2), add))
                return ops

            lo_tree = tree(lo)
            extras = dict(extras)
            for jt in range(NJT):
                jsl = slice(jt * 128, (jt + 1) * 128)
                t, j = pthalf(jt)
                sAB = psum.tile([128, 2 * IB], f32, tag="sAB")
                nc.tensor.matmul(
                    sAB[:, 0:IB],
                    lhsT=kT_t[p][0:64, jsl],
                    rhs=qT_t[p][0:64, isl],
                    start=True, stop=True, tile_position=(0, 0))
                nc.tensor.matmul(
                    sAB[:, IB:2 * IB],
                    lhsT=kT_t[p][64:128, jsl],
                    rhs=qT_t[p][64:128, isl],
                    start=True, stop=True, tile_position=(64, 0))
                nc.scalar.activation(
                    t[:, 2 * j:2 * j + 2, :].rearrange("p a b -> p (a b)"),
                    sAB, Exp, scale=SCALE)
                if jt >= 1:
                    emit_pv(jt - 1)
                if jt in extras:
                    extras.pop(jt)()
                if jt in (10, 12, 14):
                    lo_tree.pop(0)()
            emit_pv(NJT - 1)
            for op in tree(hi):
                op()
            acc = spool.tile([128, 2, IB], f32, tag="acc")
            nc.vector.tensor_tensor(
                acc.rearrange("p a b -> p (a b)"),
                lo[:, 0:2, :].rearrange("p a b -> p (a b)"),
                hi[:, 0:2, :].rearrange("p a b -> p (a b)"), add)
            dn = spool.tile([128, 2, IB], f32, tag="dn")
            nc.gpsimd.partition_all_reduce(
                dn.rearrange("p a b -> p (a b)"),
                acc.rearrange("p a b -> p (a b)"),
                channels=128, reduce_op=bass_isa.ReduceOp.add)

            def tail():
                nc.vector.reciprocal_approx_fast(
                    dn.rearrange("p a b -> p (a b)"),
                    dn.rearrange("p a b -> p (a b)"))
                oT_sb = opool.tile([128, IB], bf16, tag="oTsb")
                nc.vector.tensor_tensor(
                    oT_sb[0:64, :], oT_ps[0:64, :], dn[0:64, 0, :], mult)
                nc.vector.tensor_tensor(
                    oT_sb[64:128, :], oT_ps[64:128, :], dn[64:128, 1, :], mult)
                oT_all[(p, ib)] = oT_sb
            return tail

        # ---- prologue: qT/kT for pair 0, then V ----
        for st in qk_steps(0):
            st()
        for jt in range(NJT):
            emit_v(jt)

        # ---- main sweep: pair-outer / i-block-inner ----
        # extras injected per unit: previous unit's tail at iter 7, next
        # pair's projection bursts (p<3) or the previous i-block's output
        # projection (p==3) spread over the remaining iters.
        prev_tail = None
        for p in range(NPAIR):
            nxt = qk_steps(p + 1) if p + 1 < NPAIR else None
            for ib in range(NIB):
                extras = {}
                if prev_tail is not None:
                    extras[7] = prev_tail
                if nxt is not None:
                    for pos, st in zip((1, 3, 5, 9), nxt[4 * ib: 4 * ib + 4]):
                        extras[pos] = st
                elif ib >= 1:
                    for i, st in enumerate(outproj_steps(ib - 1)):
                        extras[8 + i] = st
                prev_tail = unit(p, ib, extras)
        prev_tail()
        for st in outproj_steps(NIB - 1):
            st()


def _prep_inputs(x, w_qkv, w_out):
    bf = ml_dtypes.bfloat16
    in_maps = []
    for c in range(8):
        b, hh = c // 2, c % 2
        xT = np.ascontiguousarray(x[b].T).astype(bf)
        q = w_qkv[:, hh * CPC:(hh + 1) * CPC]
        k = w_qkv[:, DIM + hh * CPC: DIM + (hh + 1) * CPC]
        v = w_qkv[:, 2 * DIM + hh * CPC: 2 * DIM + (hh + 1) * CPC]
        wqkv = np.ascontiguousarray(np.concatenate([q, k, v], axis=1)).astype(bf)
        wout = np.ascontiguousarray(w_out[hh * CPC:(hh + 1) * CPC, :]).astype(bf)
        in_maps.append({"xT": xT, "wqkv": wqkv, "wout": wout})
    return in_maps


def _run(x, w_qkv, w_out, b_out, trace=False):
    from concourse import bass_utils
    if "nc" not in _cache:
        _cache["nc"] = _build()
    nc = _cache["nc"]
    in_maps = _prep_inputs(x, w_qkv, w_out)
    res = bass_utils.run_bass_kernel_spmd(
        nc, in_maps, core_ids=list(range(8)), trace=trace)
    partials = [r["out"] for r in res.results]
    out = np.empty((B, N, DIM), dtype=np.float32)
    for b in range(B):
        out[b] = partials[2 * b] + partials[2 * b + 1] + b_out.astype(np.float32)
    return out, res


def kernel(x, w_qkv, w_out, b_out):
    x = np.asarray(x, dtype=np.float32)
    w_qkv = np.asarray(w_qkv, dtype=np.float32)
    w_out = np.asarray(w_out, dtype=np.float32)
    b_out = np.asarray(b_out, dtype=np.float32)
    out, _ = _run(x, w_qkv, w_out, b_out, trace=False)
    return out


# revision 18
# speedup vs baseline: 1.4969x; 1.0141x over previous
"""Multi-head attention (B=4, N=2048, DIM=1024, H=16, DH=64) on 8 trn2 cores.

Sharding: core c handles batch c//2 and head-half c%2 (8 heads).  Each core
computes qkv projection for its heads, attention, and a partial output
projection; the host sums the two partials per batch and adds the bias.
No cross-core collectives needed.

Layout strategy (zero on-device transposes):
  - host supplies x[b] pre-transposed (xT: [DIM, N]) in bf16
  - qT/kT computed as [d, n] ("transposed") via out = W^T @ x^T matmuls
  - S^T tiles [j=128, i=512] from row-packed matmuls (d=64 contraction,
    2 heads concurrently in PE row groups 0-63 / 64-127)
  - exp via ACT (scale folded), PSUM -> SBUF bf16 (P^T tiles)
  - PV: O^T[d, i] += V[j, d]^T-matmul, col-packed pairs (PE col groups)
  - denominators: DVE add-tree over j-tiles + gpsimd partition_all_reduce
  - output projection consumes O^T tiles directly as lhsT
"""

import numpy as np
import ml_dtypes

B, N, DIM = 4, 2048, 1024
HEADS, DH = 16, 64
SCALE = DIM ** (-0.5)
HPC = 8              # heads per core
NPAIR = HPC // 2     # 4 head pairs
CPC = HPC * DH       # 512 channels per core
IB = 512             # i-block (query cols per attention unit)
NIB = N // IB        # 4
NJT = N // 128       # 16 j-tiles
NKT = DIM // 128     # 8 contraction tiles for projections

_cache = {}


def _build():
    import concourse.bacc as bacc
    import concourse.mybir as mybir
    import concourse.tile as tile

    f32 = mybir.dt.float32
    bf16 = mybir.dt.bfloat16

    nc = bacc.Bacc("TRN2", target_bir_lowering=False, debug=False,
                   enable_asserts=False, num_devices=8)

    xT_d = nc.dram_tensor("xT", (DIM, N), bf16, kind="ExternalInput").ap()
    wqkv_d = nc.dram_tensor("wqkv", (DIM, 3 * CPC), bf16, kind="ExternalInput").ap()
    wout_d = nc.dram_tensor("wout", (CPC, DIM), bf16, kind="ExternalInput").ap()
    out_d = nc.dram_tensor("out", (N, DIM), f32, kind="ExternalOutput").ap()

    with tile.TileContext(nc) as tc:
        _body(nc, tc, mybir, xT_d, wqkv_d, wout_d, out_d)

    nc.compile()
    return nc


def _body(nc, tc, mybir, xT_d, wqkv_d, wout_d, out_d):
    import concourse.bass_isa as bass_isa
    from contextlib import ExitStack

    f32 = mybir.dt.float32
    bf16 = mybir.dt.bfloat16
    Exp = mybir.ActivationFunctionType.Exp
    mult = mybir.AluOpType.mult
    add = mybir.AluOpType.add
    NJH = NJT // 2   # j-tiles per half (8)

    ctx = ExitStack()
    with ctx:
        wpool = ctx.enter_context(tc.tile_pool(name="weights", bufs=1))
        qkv_pool = ctx.enter_context(tc.tile_pool(name="qkv", bufs=1))
        ppool = ctx.enter_context(tc.tile_pool(name="ptiles", bufs=2))
        ppool1 = ctx.enter_context(tc.tile_pool(name="ptiles1", bufs=1))
        spool = ctx.enter_context(tc.tile_pool(name="small", bufs=2))
        outp = ctx.enter_context(tc.tile_pool(name="outstage", bufs=3))
        opool = ctx.enter_context(tc.tile_pool(name="oT", bufs=16))
        psum = ctx.enter_context(tc.tile_pool(name="psum", bufs=2, space="PSUM"))

        # ---- weights + xT load (per k-tile so compute starts early) ----
        wqkv_sb = wpool.tile([128, NKT, 3 * CPC], bf16)
        wqkv_r = wqkv_d.rearrange("(ko p) c -> p ko c", p=128)
        for kt in range(NKT):
            nc.sync.dma_start(wqkv_sb[:, kt, :], wqkv_r[:, kt, :])
        wout_sb = wpool.tile([128, NPAIR, DIM], bf16)
        nc.sync.dma_start(wout_sb, wout_d.rearrange("(po p) n -> p po n", p=128))
        xT_sb = wpool.tile([128, NKT, N], bf16)
        xT_r = xT_d.rearrange("(ko p) n -> p ko n", p=128)
        for kt in range(NKT):
            nc.sync.dma_start(xT_sb[:, kt, :], xT_r[:, kt, :])

        # per-pair q/k tiles (separate tiles => clean dependency tracking
        # when later pairs' projections interleave into attention units)
        qT_t = [qkv_pool.tile([128, N], bf16, tag=f"qT{p}", name=f"qT{p}") for p in range(NPAIR)]
        kT_t = [qkv_pool.tile([128, N], bf16, tag=f"kT{p}", name=f"kT{p}") for p in range(NPAIR)]
        v_sb = qkv_pool.tile([128, NJT, CPC], bf16)

        # ---- emit helpers ----
        def qk_steps(p):
            """Projection of qT/kT for pair p as a list of small PE bursts."""
            steps = []
            for qk, dst in ((0, qT_t[p]), (1, kT_t[p])):
                woff = qk * CPC + p * 128
                for ib in range(NIB):
                    cell = {}

                    def stepA(cell=cell, woff=woff, ib=ib):
                        cell["ps"] = psum.tile([128, IB], f32, tag="qkvps", name="qkps")
                        for kt in range(4):
                            nc.tensor.matmul(
                                cell["ps"],
                                lhsT=wqkv_sb[:, kt, woff:woff + 128],
                                rhs=xT_sb[:, kt, ib * IB:(ib + 1) * IB],
                                start=(kt == 0), stop=False)

                    def stepB(cell=cell, woff=woff, ib=ib, dst=dst):
                        for kt in range(4, NKT):
                            nc.tensor.matmul(
                                cell["ps"],
                                lhsT=wqkv_sb[:, kt, woff:woff + 128],
                                rhs=xT_sb[:, kt, ib * IB:(ib + 1) * IB],
                                start=False, stop=(kt == NKT - 1))
                        nc.vector.tensor_copy(
                            out=dst[:, ib * IB:(ib + 1) * IB], in_=cell["ps"])

                    steps += [stepA, stepB]
            return steps

        def emit_v(jt):
            ps = psum.tile([128, CPC], f32, tag="qkvps")
            for kt in range(NKT):
                nc.tensor.matmul(
                    ps,
                    lhsT=xT_sb[:, kt, jt * 128:(jt + 1) * 128],
                    rhs=wqkv_sb[:, kt, 2 * CPC:3 * CPC],
                    start=(kt == 0), stop=(kt == NKT - 1))
            nc.vector.tensor_copy(out=v_sb[:, jt, :], in_=ps)

        oT_all = {}

        def outproj_steps(ib):
            steps = []
            for isub in range(4):
                for nh in range(2):
                    def step(isub=isub, nh=nh, ib=ib):
                        ops = psum.tile([128, 512], f32, tag="qkvps")
                        for p in range(NPAIR):
                            nc.tensor.matmul(
                                ops,
                                lhsT=oT_all[(p, ib)][:, isub * 128:(isub + 1) * 128],
                                rhs=wout_sb[:, p, nh * 512:(nh + 1) * 512],
                                start=(p == 0), stop=(p == NPAIR - 1))
                        ost = outp.tile([128, 512], f32, tag="ost")
                        nc.vector.tensor_copy(out=ost, in_=ops)
                        nc.sync.dma_start(
                            out_d[ib * IB + isub * 128: ib * IB + (isub + 1) * 128,
                                  nh * 512:(nh + 1) * 512], ost)
                    steps.append(step)
            return steps

        # ---- attention unit ----
        # P^T for a unit lives in two half tiles (j-tiles 0-7 / 8-15), each
        # [128, 2*NJH, IB] bf16 with planes indexed 2*jt_local + head.
        # Denominator add-tree runs in-place per half; the lo-half tree is
        # emitted inside the j-loop once PV consumed it.  The PAR-dependent
        # tail (reciprocal + normalize) is returned as a closure and
        # injected into the NEXT unit's j-loop (hides gpsimd latency from
        # the in-order DVE queue).
        def unit(p, ib, extras):
            isl = slice(ib * IB, (ib + 1) * IB)
            lo = ppool.tile([128, 2 * NJH, IB], bf16, tag="ptlo")
            hi = ppool1.tile([128, 2 * NJH, IB], bf16, tag="pthi")
            oT_ps = psum.tile([128, IB], f32, tag="oT")

            def pthalf(jt):
                return (lo, jt) if jt < NJH else (hi, jt - NJH)

            def emit_pv(jt):
                t, j = pthalf(jt)
                st = (jt == 0)
                sp = (jt == NJT - 1)
                nc.tensor.matmul(
                    oT_ps[0:64, :],
                    lhsT=v_sb[:, jt, (2 * p) * DH:(2 * p + 1) * DH],
                    rhs=t[:, 2 * j, :],
                    start=st, stop=sp, tile_position=(0, 0))
                nc.tensor.matmul(
                    oT_ps[64:128, :],
                    lhsT=v_sb[:, jt, (2 * p + 1) * DH:(2 * p + 2) * DH],
                    rhs=t[:, 2 * j + 1, :],
                    start=st, stop=sp, tile_position=(0, 64))

            def tree(t, n_ops=3):
                def half(lo_, n):
                    return t[:, lo_:lo_ + n, :].rearrange("p a b -> p (a b)")
                ops = []
                ops.append(lambda: nc.vector.tensor_tensor(
                    half(0, 8), half(0, 8), half(8, 8), add))
                ops.append(lambda: nc.vector.tensor_tensor(
                    half(0, 4), half(0, 4), half(4, 4), add))
                ops.append(lambda: nc.vector.tensor_tensor(
                    half(0, 2), half(0, 2), half(2, 2), add))
                return ops

            lo_tree = tree(lo)
            extras = dict(extras)
            for jt in range(NJT):
                jsl = slice(jt * 128, (jt + 1) * 128)
                t, j = pthalf(jt)
                sAB = psum.tile([128, 2 * IB], f32, tag="sAB")
                nc.tensor.matmul(
                    sAB[:, 0:IB],
                    lhsT=kT_t[p][0:64, jsl],
                    rhs=qT_t[p][0:64, isl],
                    start=True, stop=True, tile_position=(0, 0))
                nc.tensor.matmul(
                    sAB[:, IB:2 * IB],
                    lhsT=kT_t[p][64:128, jsl],
                    rhs=qT_t[p][64:128, isl],
                    start=True, stop=True, tile_position=(64, 0))
                nc.scalar.activation(
                    t[:, 2 * j:2 * j + 2, :].rearrange("p a b -> p (a b)"),
                    sAB, Exp, scale=SCALE)
                if jt >= 1:
                    emit_pv(jt - 1)
                for fn in extras.pop(jt, ()):
                    fn()
                if jt in (9, 11, 13):
                    lo_tree.pop(0)()
            emit_pv(NJT - 1)
            for op in tree(hi):
                op()
            acc = spool.tile([128, 2, IB], f32, tag="acc")
            nc.vector.tensor_tensor(
                acc.rearrange("p a b -> p (a b)"),
                lo[:, 0:2, :].rearrange("p a b -> p (a b)"),
                hi[:, 0:2, :].rearrange("p a b -> p (a b)"), add)
            dn = spool.tile([128, 2, IB], f32, tag="dn")
            nc.gpsimd.partition_all_reduce(
                dn.rearrange("p a b -> p (a b)"),
                acc.rearrange("p a b -> p (a b)"),
                channels=128, reduce_op=bass_isa.ReduceOp.add)

            def tail():
                nc.vector.reciprocal_approx_fast(
                    dn.rearrange("p a b -> p (a b)"),
                    dn.rearrange("p a b -> p (a b)"))
                oT_sb = opool.tile([128, IB], bf16, tag="oTsb")
                nc.vector.tensor_tensor(
                    oT_sb[0:64, :], oT_ps[0:64, :], dn[0:64, 0, :], mult)
                nc.vector.tensor_tensor(
                    oT_sb[64:128, :], oT_ps[64:128, :], dn[64:128, 1, :], mult)
                oT_all[(p, ib)] = oT_sb
            return tail

        # ---- prologue: qT/kT for pair 0, then V ----
        for st in qk_steps(0):
            st()
        for jt in range(NJT):
            emit_v(jt)

        # ---- main sweep: pair-outer / i-block-inner ----
        # extras injected per unit: previous unit's tail at iter 7, next
        # pair's projection bursts (p<3) or the previous i-block's output
        # projection (p==3) spread over the remaining iters.
        prev_tail = None
        for p in range(NPAIR):
            nxt = qk_steps(p + 1) if p + 1 < NPAIR else None
            for ib in range(NIB):
                extras = {}
                if prev_tail is not None:
                    extras[10] = [prev_tail]
                if nxt is not None:
                    for pos, st in zip((1, 2, 3, 4, 5, 6), nxt[6 * ib: 6 * ib + 6]):
                        extras.setdefault(pos, []).append(st)
                elif ib >= 1:
                    ops_ = outproj_steps(ib - 1)
                    for pos, group in zip((11, 12, 13, 14, 15),
                                          (ops_[0:2], ops_[2:4], ops_[4:6],
                                           ops_[6:7], ops_[7:8])):
                        extras.setdefault(pos, []).extend(group)
                prev_tail = unit(p, ib, extras)
        prev_tail()
        for st in outproj_steps(NIB - 1):
            st()


def _prep_inputs(x, w_qkv, w_out):
    bf = ml_dtypes.bfloat16
    in_maps = []
    for c in range(8):
        b, hh = c // 2, c % 2
        xT = np.ascontiguousarray(x[b].T).astype(bf)
        q = w_qkv[:, hh * CPC:(hh + 1) * CPC]
        k = w_qkv[:, DIM + hh * CPC: DIM + (hh + 1) * CPC]
        v = w_qkv[:, 2 * DIM + hh * CPC: 2 * DIM + (hh + 1) * CPC]
        wqkv = np.ascontiguousarray(np.concatenate([q, k, v], axis=1)).astype(bf)
        wout = np.ascontiguousarray(w_out[hh * CPC:(hh + 1) * CPC, :]).astype(bf)
        in_maps.append({"xT": xT, "wqkv": wqkv, "wout": wout})
    return in_maps


def _run(x, w_qkv, w_out, b_out, trace=False):
    from concourse import bass_utils
    if "nc" not in _cache:
        _cache["nc"] = _build()
    nc = _cache["nc"]
    in_maps = _prep_inputs(x, w_qkv, w_out)
    res = bass_utils.run_bass_kernel_spmd(
        nc, in_maps, core_ids=list(range(8)), trace=trace)
    partials = [r["out"] for r in res.results]
    out = np.empty((B, N, DIM), dtype=np.float32)
    for b in range(B):
        out[b] = partials[2 * b] + partials[2 * b + 1] + b_out.astype(np.float32)
    return out, res


def kernel(x, w_qkv, w_out, b_out):
    x = np.asarray(x, dtype=np.float32)
    w_qkv = np.asarray(w_qkv, dtype=np.float32)
    w_out = np.asarray(w_out, dtype=np.float32)
    b_out = np.asarray(b_out, dtype=np.float32)
    out, _ = _run(x, w_qkv, w_out, b_out, trace=False)
    return out
